# revision 2
# baseline (speedup 1.0000x reference)
"""DeepPheno model kernel for 8 TRN2 NeuronCores: two-phase SPMD pipeline.

Computation (reference):
    h    = gelu(gos @ W1 + b1)                     (B, HID)     erf-gelu
    x    = concat([h, exp_x], 1)                   (B, HID+EXP)
    flat = sigmoid(x @ W2 + b2)                    (B, C)
    out  = max_i flat[b, j] * M[i, j]              (B, C)

Since flat = sigmoid(..) > 0, the max-pool factorizes exactly:
    out[b, j] = flat[b, j] * max_i M[i, j].

Why two phases: the only cross-core dataflow is the gather of h between
the two matmuls.  On this stack an on-device ncfw collective costs
~75us from its first doorbell (entry barrier ~47.5us + fixed ~11us
post-barrier gap + per-op costs), which floors any single-NEFF design
at ~95us regardless of DMA/compute optimization.  Splitting at the
gather instead:

  Phase 1 (tensor-parallel over HID): core c computes its 192 of 1536
      (padded) hidden columns: h_c = gelu(gos @ W1_c), emitted
      batch-major (64, 192) fp16.  b1 is folded into the matmul via an
      augmented ones-row of gos.T at k=10000 (inside the k-padding), so
      the matmul+bias+gelu is a single PSUM->ACT pass.  One
      LDWEIGHTS+MATMUL pair per k-tile (lhsT=gos tile, 192-wide moving
      W1 tile).
  Host: concatenates the 8 h-shards + exp_x into the x.T image (pure
      layout, part of the unshard/reshard the harness permits).
  Phase 2 (tensor-parallel over classes): core c computes its 256
      classes: sigmoid(x @ W2_c + b2_c) * colmax(M_c), with the colmax
      reduced on-device from M's column shard.

All large operands are host-cast fp16 (rel err ~1.6e-4); biases and the
output stay fp32.  Per-core HBM traffic: phase 1 = 5.05MB, phase 2 =
2.4MB; weights are read by exactly one core; only gos (1.26MB fp16) is
replicated.  Measured: ~32us + ~24us = ~56us total vs 95.5us for the
single-NEFF AllGather baseline.
"""

import numpy as np

import concourse.bacc as bacc
import concourse.mybir as mybir
import concourse.tile as tile
from concourse.bass_utils import run_bass_kernel_spmd

B = 64
IN = 10000
EXP = 53
HID = 1500
C = 2048

NCORES = 8
HD = 192            # hid columns per core (1536 = 8*192 padded)
HIDP = HD * NCORES
CD = C // NCORES    # 256 classes per core
KT1 = 79            # 79*128 = 10112 >= 10001 (gos rows + ones row)
K1P = KT1 * 128
KT2 = 17            # 17*128 = 2176 >= 1536 + 53
K2P = KT2 * 128

F32 = mybir.dt.float32
F16 = mybir.dt.float16

CHB1 = [0, 10, 20, 30, 40, 50, 60, 70, KT1]   # phase-1 k chunks


def _build_p1():
    nc = bacc.Bacc(
        "TRN2",
        target_bir_lowering=False,
        debug=False,
        enable_asserts=False,
        num_devices=NCORES,
    )
    gos_d = nc.dram_tensor("gos_img", [128, KT1 * B], F16, kind="ExternalInput")
    w1_d = nc.dram_tensor("w1_img", [128, KT1 * HD], F16, kind="ExternalInput")
    h_d = nc.dram_tensor("h_img", [B, HD], F16, kind="ExternalOutput")

    with tile.TileContext(nc) as tc:
        with (
            tc.tile_pool(name="persist", bufs=1) as pp,
            tc.tile_pool(name="small", bufs=1) as sp,
            tc.tile_pool(name="psum", bufs=1, space="PSUM") as psp,
        ):
            gos_sb = pp.tile([128, KT1 * B], F16, tag="gos")
            w1_sb = pp.tile([128, KT1 * HD], F16, tag="w1")
            # single ring, k-interleaved so the PE is fed in order
            for a, b in zip(CHB1[:-1], CHB1[1:]):
                nc.sync.dma_start(
                    out=gos_sb[:, a * B : b * B], in_=gos_d[:, a * B : b * B]
                )
                nc.sync.dma_start(
                    out=w1_sb[:, a * HD : b * HD], in_=w1_d[:, a * HD : b * HD]
                )

            hP = psp.tile([B, HD], F32, tag="hP")
            for n in range(KT1):
                nc.tensor.matmul(
                    hP[:, :],
                    lhsT=gos_sb[:, n * B : (n + 1) * B],
                    rhs=w1_sb[:, n * HD : (n + 1) * HD],
                    start=(n == 0),
                    stop=(n == KT1 - 1),
                )

            h16 = sp.tile([B, HD], F16, tag="h16")
            nc.scalar.activation(
                h16[:, :], hP[:, :], mybir.ActivationFunctionType.Gelu
            )
            nc.sync.dma_start(out=h_d[:, :], in_=h16[:, :])

    nc.compile()
    return nc


def _build_p2():
    nc = bacc.Bacc(
        "TRN2",
        target_bir_lowering=False,
        debug=False,
        enable_asserts=False,
        num_devices=NCORES,
    )
    x_d = nc.dram_tensor("x_img", [128, KT2 * B], F16, kind="ExternalInput")
    w2_d = nc.dram_tensor("w2_img", [128, 2 * KT2 * 128], F16, kind="ExternalInput")
    mt_d = nc.dram_tensor("mt_img", [128, 2 * C], F16, kind="ExternalInput")
    b2_d = nc.dram_tensor("b2_img", [128, 2], F32, kind="ExternalInput")
    out_d = nc.dram_tensor("out_img", [128, 2 * B], F32, kind="ExternalOutput")

    with tile.TileContext(nc) as tc:
        with (
            tc.tile_pool(name="persist", bufs=1) as pp,
            tc.tile_pool(name="small", bufs=1) as sp,
            tc.tile_pool(name="psum", bufs=1, space="PSUM") as psp,
        ):
            b2_sb = sp.tile([128, 2], F32, tag="b2")
            nc.scalar.dma_start(out=b2_sb[:, :], in_=b2_d[:, :])

            # sync ring: x then W2 (mm2 operands); scalar ring: M (colmax)
            x_sb = pp.tile([128, KT2 * B], F16, tag="x")
            nc.sync.dma_start(out=x_sb[:, :], in_=x_d[:, :])
            w2_sb = pp.tile([128, 2 * KT2 * 128], F16, tag="w2")
            for mb in range(2):
                for half in range(2):
                    sl = slice(
                        (2 * mb + half) * (KT2 * 64),
                        (2 * mb + half + 1) * (KT2 * 64),
                    )
                    nc.sync.dma_start(out=w2_sb[:, sl], in_=w2_d[:, sl])

            mt_sb = pp.tile([128, 2 * C], F16, tag="mt")
            cm_sb = sp.tile([128, 2], F32, tag="cm")
            for mb in range(2):
                sl = slice(mb * C, (mb + 1) * C)
                nc.scalar.dma_start(out=mt_sb[:, sl], in_=mt_d[:, sl])
                nc.vector.reduce_max(
                    cm_sb[:, mb : mb + 1], mt_sb[:, sl], axis=mybir.AxisListType.X
                )

            o_sb = sp.tile([128, 2 * B], F32, tag="osb")
            for mb in range(2):
                pf = psp.tile([128, B], F32, tag=f"pf{mb}")
                base = mb * KT2 * 128
                for t in range(KT2):
                    nc.tensor.matmul(
                        pf[:, :],
                        lhsT=w2_sb[:, base + t * 128 : base + (t + 1) * 128],
                        rhs=x_sb[:, t * B : (t + 1) * B],
                        start=(t == 0),
                        stop=(t == KT2 - 1),
                    )
                f_sb = sp.tile([128, B], F32, tag=f"f{mb}")
                nc.scalar.activation(
                    f_sb[:, :], pf[:, :],
                    mybir.ActivationFunctionType.Sigmoid,
                    bias=b2_sb[:, mb : mb + 1],
                )
                nc.vector.tensor_scalar_mul(
                    o_sb[:, mb * B : (mb + 1) * B], f_sb[:, :],
                    cm_sb[:, mb : mb + 1],
                )
                eng = nc.sync if mb == 0 else nc.scalar
                eng.dma_start(
                    out=out_d[:, mb * B : (mb + 1) * B],
                    in_=o_sb[:, mb * B : (mb + 1) * B],
                )

    nc.compile()
    return nc


_NC1 = None
_NC2 = None


def _get_ncs():
    global _NC1, _NC2
    if _NC1 is None:
        _NC1 = _build_p1()
        _NC2 = _build_p2()
    return _NC1, _NC2


def _tile_img(arr2d, ktiles):
    """(ktiles*128, m) -> SBUF image (128, ktiles*m), k-tile-major free dim."""
    k, m = arr2d.shape
    assert k == ktiles * 128
    return np.ascontiguousarray(
        arr2d.reshape(ktiles, 128, m).transpose(1, 0, 2).reshape(128, ktiles * m)
    )


def _prep_p1(gos, W1, b1):
    f16 = np.float16
    gosT = np.zeros((K1P, B), f16)
    gosT[:IN] = np.asarray(gos, np.float32).T.astype(f16)
    gosT[IN] = 1.0  # ones row: multiplies the bias row of W1
    gos_img = _tile_img(gosT, KT1)

    W1p = np.zeros((K1P, HIDP), f16)
    W1p[:IN, :HID] = np.asarray(W1, np.float32).astype(f16)
    W1p[IN, :HID] = np.asarray(b1, np.float32).astype(f16)

    in_maps = []
    for c in range(NCORES):
        w1c = W1p[:, HD * c : HD * (c + 1)]
        in_maps.append(
            {
                "gos_img": gos_img,
                "w1_img": np.ascontiguousarray(_tile_img(w1c, KT1)),
            }
        )
    return in_maps


def _prep_p2(h_shards, exp_x, W2, b2, hpo_matrix):
    f16 = np.float16
    f32 = np.float32
    # gather: x.T = [h.T (1536 rows incl pad) | exp.T | zero pad]
    xT = np.zeros((K2P, B), f16)
    for c, h in enumerate(h_shards):
        xT[HD * c : HD * (c + 1)] = np.asarray(h).T  # (192, 64) fp16
    xT[HIDP : HIDP + EXP] = np.asarray(exp_x, f32).T.astype(f16)
    x_img = _tile_img(xT, KT2)

    W2 = np.asarray(W2, f32)
    W2p = np.zeros((K2P, C), f16)
    W2p[:HID] = W2[:HID].astype(f16)
    W2p[HIDP : HIDP + EXP] = W2[HID:].astype(f16)
    b2 = np.asarray(b2, f32)
    M16 = np.asarray(hpo_matrix, f32).astype(f16)

    in_maps = []
    for c in range(NCORES):
        c0, c1 = CD * c, CD * (c + 1)
        w2c = W2p[:, c0:c1]
        w2_img = np.concatenate(
            [_tile_img(w2c[:, :128], KT2), _tile_img(w2c[:, 128:], KT2)], axis=1
        )
        mt = np.ascontiguousarray(M16[:, c0:c1].T)  # (256, 2048)
        mt_img = np.concatenate([mt[:128], mt[128:]], axis=1)
        b2_img = np.zeros((128, 2), f32)
        b2_img[:, 0] = b2[c0 : c0 + 128]
        b2_img[:, 1] = b2[c0 + 128 : c1]
        in_maps.append(
            {
                "x_img": x_img,
                "w2_img": np.ascontiguousarray(w2_img),
                "mt_img": np.ascontiguousarray(mt_img),
                "b2_img": b2_img,
            }
        )
    return in_maps


def _assemble_output(results):
    cols = []
    for r in results:
        o = r["out_img"]  # (128, 2B): [p, mb*B + b] = flat.T[mb*128+p, b]*cm
        chunk = o.reshape(128, 2, B).transpose(1, 0, 2).reshape(CD, B)
        cols.append(chunk.T)
    return np.ascontiguousarray(np.concatenate(cols, axis=1))


def kernel(gos, exp_x, W1, b1, W2, b2, hpo_matrix, **kw):
    nc1, nc2 = _get_ncs()
    in1 = _prep_p1(gos, W1, b1)
    res1 = run_bass_kernel_spmd(nc1, in1, core_ids=list(range(NCORES)))
    h_shards = [r["h_img"] for r in res1.results]
    in2 = _prep_p2(h_shards, exp_x, W2, b2, hpo_matrix)
    res2 = run_bass_kernel_spmd(nc2, in2, core_ids=list(range(NCORES)))
    return _assemble_output(res2.results)


# revision 4
# speedup vs baseline: 1.0201x; 1.0201x over previous
"""DeepPheno model kernel for 8 TRN2 NeuronCores: two-phase SPMD pipeline.

Computation (reference):
    h    = gelu(gos @ W1 + b1)                     (B, HID)     erf-gelu
    x    = concat([h, exp_x], 1)                   (B, HID+EXP)
    flat = sigmoid(x @ W2 + b2)                    (B, C)
    out  = max_i flat[b, j] * M[i, j]              (B, C)

Since flat = sigmoid(..) > 0, the max-pool factorizes exactly:
    out[b, j] = flat[b, j] * max_i M[i, j].

Why two phases: the only cross-core dataflow is the gather of h between
the two matmuls.  On this stack an on-device ncfw collective costs
~75us from its first doorbell (entry barrier ~47.5us + fixed ~11us
post-barrier gap + per-op costs), which floors any single-NEFF design
at ~95us regardless of DMA/compute optimization.  Splitting at the
gather instead:

  Phase 1 (tensor-parallel over HID): core c computes its 192 of 1536
      (padded) hidden columns: h_c = gelu(gos @ W1_c), emitted
      batch-major (64, 192) fp16.  b1 is folded into the matmul via an
      augmented ones-row of gos.T at k=10000 (inside the k-padding), so
      the matmul+bias+gelu is a single PSUM->ACT pass.  One
      LDWEIGHTS+MATMUL pair per k-tile (lhsT=gos tile, 192-wide moving
      W1 tile).
  Host: concatenates the 8 h-shards + exp_x into the x.T image (pure
      layout, part of the unshard/reshard the harness permits).
  Phase 2 (tensor-parallel over classes): core c computes its 256
      classes: sigmoid(x @ W2_c + b2_c) * colmax(M_c), with the colmax
      reduced on-device from M's column shard.

All large operands are host-cast fp16 (rel err ~1.6e-4); biases and the
output stay fp32.  Per-core HBM traffic: phase 1 = 5.05MB, phase 2 =
2.4MB; weights are read by exactly one core; only gos (1.26MB fp16) is
replicated.  Measured: ~32us + ~24us = ~56us total vs 95.5us for the
single-NEFF AllGather baseline.
"""

import numpy as np

import concourse.bacc as bacc
import concourse.mybir as mybir
import concourse.tile as tile
from concourse.bass_utils import run_bass_kernel_spmd

B = 64
IN = 10000
EXP = 53
HID = 1500
C = 2048

NCORES = 8
HD = 192            # hid columns per core (1536 = 8*192 padded)
HIDP = HD * NCORES
CD = C // NCORES    # 256 classes per core
KT1 = 79            # 79*128 = 10112 >= 10001 (gos rows + ones row)
K1P = KT1 * 128
KT2 = 17            # 17*128 = 2176 >= 1536 + 53
K2P = KT2 * 128

F32 = mybir.dt.float32
F16 = mybir.dt.float16

CHB1 = [0, 10, 20, 30, 40, 50, 60, 70, KT1]   # phase-1 k chunks


def _build_p1():
    nc = bacc.Bacc(
        "TRN2",
        target_bir_lowering=False,
        debug=False,
        enable_asserts=False,
        num_devices=NCORES,
    )
    gos_d = nc.dram_tensor("gos_img", [128, KT1 * B], F16, kind="ExternalInput")
    w1_d = nc.dram_tensor("w1_img", [128, KT1 * HD], F16, kind="ExternalInput")
    h_d = nc.dram_tensor("h_img", [B, HD], F16, kind="ExternalOutput")

    with tile.TileContext(nc) as tc:
        with (
            tc.tile_pool(name="persist", bufs=1) as pp,
            tc.tile_pool(name="small", bufs=1) as sp,
            tc.tile_pool(name="psum", bufs=1, space="PSUM") as psp,
        ):
            gos_sb = pp.tile([128, KT1 * B], F16, tag="gos")
            w1_sb = pp.tile([128, KT1 * HD], F16, tag="w1")
            # single ring, k-interleaved so the PE is fed in order
            for a, b in zip(CHB1[:-1], CHB1[1:]):
                nc.sync.dma_start(
                    out=gos_sb[:, a * B : b * B], in_=gos_d[:, a * B : b * B]
                )
                nc.sync.dma_start(
                    out=w1_sb[:, a * HD : b * HD], in_=w1_d[:, a * HD : b * HD]
                )

            hP = psp.tile([B, HD], F32, tag="hP")
            for n in range(KT1):
                nc.tensor.matmul(
                    hP[:, :],
                    lhsT=gos_sb[:, n * B : (n + 1) * B],
                    rhs=w1_sb[:, n * HD : (n + 1) * HD],
                    start=(n == 0),
                    stop=(n == KT1 - 1),
                )

            h16 = sp.tile([B, HD], F16, tag="h16")
            nc.scalar.activation(
                h16[:, :], hP[:, :], mybir.ActivationFunctionType.Gelu
            )
            nc.sync.dma_start(out=h_d[:, :], in_=h16[:, :])

    nc.compile()
    return nc


def _build_p2():
    nc = bacc.Bacc(
        "TRN2",
        target_bir_lowering=False,
        debug=False,
        enable_asserts=False,
        num_devices=NCORES,
    )
    x_d = nc.dram_tensor("x_img", [128, KT2 * B], F16, kind="ExternalInput")
    w2_d = nc.dram_tensor("w2_img", [128, 2 * KT2 * 128], F16, kind="ExternalInput")
    mt_d = nc.dram_tensor("mt_img", [128, 2 * C], F16, kind="ExternalInput")
    b2_d = nc.dram_tensor("b2_img", [128, 2], F32, kind="ExternalInput")
    out_d = nc.dram_tensor("out_img", [128, 2 * B], F32, kind="ExternalOutput")

    with tile.TileContext(nc) as tc:
        with (
            tc.tile_pool(name="persist", bufs=1) as pp,
            tc.tile_pool(name="small", bufs=1) as sp,
            tc.tile_pool(name="psum", bufs=1, space="PSUM") as psp,
        ):
            b2_sb = sp.tile([128, 2], F32, tag="b2")
            nc.scalar.dma_start(out=b2_sb[:, :], in_=b2_d[:, :])

            # sync ring: x then W2 (mm2 operands); scalar ring: M (colmax)
            x_sb = pp.tile([128, KT2 * B], F16, tag="x")
            nc.sync.dma_start(out=x_sb[:, :], in_=x_d[:, :])
            w2_sb = pp.tile([128, 2 * KT2 * 128], F16, tag="w2")
            for mb in range(2):
                for half in range(2):
                    sl = slice(
                        (2 * mb + half) * (KT2 * 64),
                        (2 * mb + half + 1) * (KT2 * 64),
                    )
                    nc.sync.dma_start(out=w2_sb[:, sl], in_=w2_d[:, sl])

            mt_sb = pp.tile([128, 2 * C], F16, tag="mt")
            cm_sb = sp.tile([128, 2], F32, tag="cm")
            for mb in range(2):
                sl = slice(mb * C, (mb + 1) * C)
                nc.scalar.dma_start(out=mt_sb[:, sl], in_=mt_d[:, sl])
                nc.vector.reduce_max(
                    cm_sb[:, mb : mb + 1], mt_sb[:, sl], axis=mybir.AxisListType.X
                )

            o_sb = sp.tile([128, 2 * B], F32, tag="osb")
            for mb in range(2):
                pf = psp.tile([128, B], F32, tag=f"pf{mb}")
                base = mb * KT2 * 128
                for t in range(KT2):
                    nc.tensor.matmul(
                        pf[:, :],
                        lhsT=w2_sb[:, base + t * 128 : base + (t + 1) * 128],
                        rhs=x_sb[:, t * B : (t + 1) * B],
                        start=(t == 0),
                        stop=(t == KT2 - 1),
                    )
                f_sb = sp.tile([128, B], F32, tag=f"f{mb}")
                nc.scalar.activation(
                    f_sb[:, :], pf[:, :],
                    mybir.ActivationFunctionType.Sigmoid,
                    bias=b2_sb[:, mb : mb + 1],
                )
                nc.vector.tensor_scalar_mul(
                    o_sb[:, mb * B : (mb + 1) * B], f_sb[:, :],
                    cm_sb[:, mb : mb + 1],
                )
                eng = nc.sync if mb == 0 else nc.scalar
                eng.dma_start(
                    out=out_d[:, mb * B : (mb + 1) * B],
                    in_=o_sb[:, mb * B : (mb + 1) * B],
                )

    nc.compile()
    return nc


_NC1 = None
_NC2 = None


def _get_ncs():
    global _NC1, _NC2
    if _NC1 is None:
        _NC1 = _build_p1()
        _NC2 = _build_p2()
    return _NC1, _NC2


def _tile_img(arr2d, ktiles):
    """(ktiles*128, m) -> SBUF image (128, ktiles*m), k-tile-major free dim."""
    k, m = arr2d.shape
    assert k == ktiles * 128
    return np.ascontiguousarray(
        arr2d.reshape(ktiles, 128, m).transpose(1, 0, 2).reshape(128, ktiles * m)
    )


def _prep_p1(gos, W1, b1):
    f16 = np.float16
    gosT = np.zeros((K1P, B), f16)
    gosT[:IN] = np.asarray(gos, np.float32).T.astype(f16)
    gosT[IN] = 1.0  # ones row: multiplies the bias row of W1
    gos_img = _tile_img(gosT, KT1)

    W1p = np.zeros((K1P, HIDP), f16)
    W1p[:IN, :HID] = np.asarray(W1, np.float32).astype(f16)
    W1p[IN, :HID] = np.asarray(b1, np.float32).astype(f16)

    in_maps = []
    for c in range(NCORES):
        w1c = W1p[:, HD * c : HD * (c + 1)]
        in_maps.append(
            {
                "gos_img": gos_img,
                "w1_img": np.ascontiguousarray(_tile_img(w1c, KT1)),
            }
        )
    return in_maps


def _prep_p2(h_shards, exp_x, W2, b2, hpo_matrix):
    f16 = np.float16
    f32 = np.float32
    # gather: x.T = [h.T (1536 rows incl pad) | exp.T | zero pad]
    xT = np.zeros((K2P, B), f16)
    for c, h in enumerate(h_shards):
        xT[HD * c : HD * (c + 1)] = np.asarray(h).T  # (192, 64) fp16
    xT[HIDP : HIDP + EXP] = np.asarray(exp_x, f32).T.astype(f16)
    x_img = _tile_img(xT, KT2)

    W2 = np.asarray(W2, f32)
    W2p = np.zeros((K2P, C), f16)
    W2p[:HID] = W2[:HID].astype(f16)
    W2p[HIDP : HIDP + EXP] = W2[HID:].astype(f16)
    b2 = np.asarray(b2, f32)
    M16 = np.asarray(hpo_matrix, f32).astype(f16)

    in_maps = []
    for c in range(NCORES):
        c0, c1 = CD * c, CD * (c + 1)
        w2c = W2p[:, c0:c1]
        w2_img = np.concatenate(
            [_tile_img(w2c[:, :128], KT2), _tile_img(w2c[:, 128:], KT2)], axis=1
        )
        mt = np.ascontiguousarray(M16[:, c0:c1].T)  # (256, 2048)
        mt_img = np.concatenate([mt[:128], mt[128:]], axis=1)
        b2_img = np.zeros((128, 2), f32)
        b2_img[:, 0] = b2[c0 : c0 + 128]
        b2_img[:, 1] = b2[c0 + 128 : c1]
        in_maps.append(
            {
                "x_img": x_img,
                "w2_img": np.ascontiguousarray(w2_img),
                "mt_img": np.ascontiguousarray(mt_img),
                "b2_img": b2_img,
            }
        )
    return in_maps


def _assemble_output(results):
    cols = []
    for r in results:
        o = r["out_img"]  # (128, 2B): [p, mb*B + b] = flat.T[mb*128+p, b]*cm
        chunk = o.reshape(128, 2, B).transpose(1, 0, 2).reshape(CD, B)
        cols.append(chunk.T)
    return np.ascontiguousarray(np.concatenate(cols, axis=1))


def kernel(gos, exp_x, W1, b1, W2, b2, hpo_matrix, **kw):
    nc1, nc2 = _get_ncs()
    in1 = _prep_p1(gos, W1, b1)
    res1 = run_bass_kernel_spmd(nc1, in1, core_ids=list(range(NCORES)))
    h_shards = [r["h_img"] for r in res1.results]
    in2 = _prep_p2(h_shards, exp_x, W2, b2, hpo_matrix)
    res2 = run_bass_kernel_spmd(nc2, in2, core_ids=list(range(NCORES)))
    return _assemble_output(res2.results)
